# revision 16
# baseline (speedup 1.0000x reference)
"""Associative-embedding (AE) loss on 8 TRN2 NeuronCores, data-parallel over batch.

Reference computation (per batch image b):
  g[m,k,:]   = tags[b, idx[b,m,k], :]                       (gather, T=8)
  mean[m,:]  = sum_k vf*g / max(cnt,1)                      (cnt = sum_k vf)
  pull       = (1/max(n,1)) * sum_m (1/max(cnt,1)) * sum_k vf * mean_t (g-mean)^2
  push       = 0.5/max(n(n-1),1) * sum_{i!=j valid} exp(-||mean_i-mean_j||^2)  (if n>1)
  out[b]     = [push, pull]

Sharding: batch dim B=64 split across 8 cores (8 images each); all reductions
are batch-local, no collectives; host concatenates per-core outputs.

Gather strategy: instead of 34 per-joint indirect DMAs (~1us of Pool SWDGE
descriptor-generation each), use TWO InstDMAGatherAnt instructions, one per
4-image quad. Each consumes int16 *block* indices and fetches the 256-byte
block (8 tag rows) containing each joint's row:
  - per-core tags are viewed as two [32768, 64] f32 halves (4 images each) so
    block indices fit int16's positive range,
  - item (slot k, partition p) of a gather lands at out[p, k, 0:64]; we place
    persons on partitions (120 of 128 used) and joints on slots,
  - the 8->1 sub-row selection is done on-chip with host-built masks
    M[p,k,s] = vf * (row & 7 == s), broadcast over the tag dim via a
    stride-0 AP, fused into one tensor_tensor_reduce per quad (gm = blk * M),
  - per-person sums then never need per-joint tensors: S1 = sum_{k,s} gm
    (DVE reduce keeping t), S2 = sum gm*blk = sum vf*g^2 (second ttr).
The remaining tail matches the old kernel: mean -> PE transpose -> Gram ->
exp(2G - r_i) with row-norm bias -> masked matmuls for push, and
pull = (S2 + cnt*rneg)/(T*cnt); n-derived factors come from the masks alone
and complete during the gather window.
"""

import numpy as np

import concourse.bass as bass
import concourse.tile as tile
from concourse import bacc, mybir
from concourse.bass_utils import run_bass_kernel_spmd
from concourse.tile_rust import add_dep_helper

B, N, T = 64, 65536, 8
M, K = 30, 17
NCORES = 8
BL = B // NCORES   # images per core
TB = 4             # images per quad
NQ = BL // TB      # quads per core (2)
P = 128            # partitions (TB*M = 120 used)
PU = TB * M        # used partitions
NI = K * P         # num_idxs per quad gather (2176)
NBLK = TB * N // 8  # 32768 blocks per tags half
F32 = mybir.dt.float32
F32R = mybir.dt.float32r
I16 = mybir.dt.int16
U8 = mybir.dt.uint8

ALU = mybir.AluOpType
AX = mybir.AxisListType
ACT = mybir.ActivationFunctionType

# packed-constant byte layout (per partition) for the two input DMAs
#   pk1: [idxA (272B) | idxB (272B)]                      -> needed first
#   pk2: [M_A (544B) | M_B (544B) | ident (512B) | bmv(16B) | ones(4B) |
#         cnt(8B) | icnt(8B) | wt(32B) | wtn(32B) | fac(64B, partition 0)]
PK1_B = 544
PK2_B = 544 + 544 + 512 + 16 + 4 + 8 + 8 + 32 + 32 + 64


def build_nc():
    nc = bacc.Bacc("TRN2", target_bir_lowering=False, debug=False, num_devices=NCORES)
    tags_a = nc.declare_dram_parameter("tags_a", [NBLK, 64], F32, isOutput=False)
    tags_b = nc.declare_dram_parameter("tags_b", [NBLK, 64], F32, isOutput=False)
    pk1_ext = nc.declare_dram_parameter("pk1", [P, PK1_B], U8, isOutput=False)
    pk2_ext = nc.declare_dram_parameter("pk2", [P, PK2_B], U8, isOutput=False)
    out_ext = nc.declare_dram_parameter("out", [1, BL * 2], F32, isOutput=True)

    with tile.TileContext(nc) as tc:
        with (
            tc.tile_pool(name="sb", bufs=1) as sb,
            tc.tile_pool(name="ps", bufs=1, space="PSUM") as ps,
        ):
            pk1 = sb.tile([P, PK1_B], U8, tag="pk1")
            nc.sync.dma_start(pk1[:], pk1_ext[:])
            pk2 = sb.tile([P, PK2_B], U8, tag="pk2")
            nc.scalar.dma_start(pk2[:], pk2_ext[:])

            idx = [pk1[:, 0:272].bitcast(I16), pk1[:, 272:544].bitcast(I16)]
            msk = [
                pk2[:, 0:544].bitcast(F32).rearrange("p (k s) -> p k s", s=8),
                pk2[:, 544:1088].bitcast(F32).rearrange("p (k s) -> p k s", s=8),
            ]
            ident = pk2[:, 1088:1600].bitcast(F32)
            bmv = pk2[:, 1600:1616].bitcast(F32)
            ones_c = pk2[:, 1616:1620].bitcast(F32)
            cnt_q = [pk2[:, 1620 + 4 * q:1624 + 4 * q].bitcast(F32) for q in range(NQ)]
            icnt_q = [pk2[:, 1628 + 4 * q:1632 + 4 * q].bitcast(F32) for q in range(NQ)]
            wt_q = [pk2[:, 1636 + 16 * q:1652 + 16 * q].bitcast(F32) for q in range(NQ)]
            wtn_q = [pk2[:, 1668 + 16 * q:1684 + 16 * q].bitcast(F32) for q in range(NQ)]
            fac = pk2[0:1, 1700:1764].bitcast(F32)

            srcs = [tags_a, tags_b]
            blk = [sb.tile([P, K, 64], F32, tag=f"blk{q}", name=f"blk{q}") for q in range(NQ)]
            gm = [sb.tile([P, K, 8, 8], F32, tag=f"gm{q}", name=f"gm{q}") for q in range(NQ)]
            sq = sb.tile([P, K * 64], F32, tag="sq")
            stats_ps = ps.tile([1, NQ * 8], F32, tag="stats", bufs=1, space="PSUM")
            out_sb = sb.tile([1, BL * 2], F32, tag="out_sb")

            # quad gathers, chunked: HW caps one dma_gather at 1024 indices,
            # so each quad is 3 instructions (slots 0-7 / 8-15 / 16).
            # Order A1 A2 B1 A3 B2 B3: quad A completes early (its whole tail
            # hides under quad B's transfers) and the bus stays near-saturated.
            CHUNKS = [(0, 0, 8), (0, 8, 16), (1, 0, 8), (1, 8, 16), (0, 16, 17), (1, 16, 17)]
            CHUNK_OFF = {(0, 8): 0, (8, 16): 128, (16, 17): 256}
            for (q, k0, k1) in CHUNKS:
                ob = 272 * q + CHUNK_OFF[(k0, k1)]
                ni = (k1 - k0) * P
                nc.gpsimd.dma_gather(
                    out_ap=blk[q][:, k0:k1, :],
                    in_ap=srcs[q][:],
                    idxs_ap=pk1[:, ob:ob + ni // 8].bitcast(I16),
                    num_idxs=ni,
                    num_idxs_reg=ni,
                    elem_size=64,
                )

            # per-quad pipeline, processed per gather chunk in arrival order so
            # the DVE/ACT streams never head-of-line block on late data
            s2p = sb.tile([P, NQ * 3], F32, tag="s2p")
            s2 = sb.tile([P, NQ], F32, tag="s2")
            s1p = [sb.tile([P, 24], F32, tag=f"s1p{q}", name=f"s1p{q}") for q in range(NQ)]
            s1 = [sb.tile([P, 8], F32, tag=f"s1{q}", name=f"s1{q}") for q in range(NQ)]
            mn = [sb.tile([P, 8], F32, tag=f"mn{q}", name=f"mn{q}") for q in range(NQ)]

            def chunk_ops(q, k0, k1, j):
                nk = k1 - k0
                gm_s = gm[q][:, k0:k1]
                nc.vector.tensor_tensor(
                    out=gm_s,
                    in0=blk[q][:, k0:k1, :].rearrange("p k (s t) -> p k s t", t=8),
                    in1=msk[q][:, k0:k1].broadcast_to((P, nk, 8, 8)),
                    op=ALU.mult,
                )
                nc.vector.reduce_sum(
                    out=s1p[q][:, 8 * j:8 * (j + 1)],
                    in_=gm_s.rearrange("p k s t -> p t (k s)"),
                    axis=AX.X,
                )
                nc.scalar.activation(
                    out=sq[:, k0 * 64:k1 * 64],
                    in_=gm_s.rearrange("p k s t -> p (k s t)"),
                    func=ACT.Square, accum_out=s2p[:, 3 * q + j:3 * q + j + 1],
                )

            def finish_core(q):
                sp = s1p[q]
                nc.vector.tensor_tensor(out=s1[q][:], in0=sp[:, 0:8], in1=sp[:, 8:16], op=ALU.add)
                nc.vector.tensor_tensor(out=s1[q][:], in0=s1[q][:], in1=sp[:, 16:24], op=ALU.add)
                nc.vector.tensor_scalar_mul(out=mn[q][:], in0=s1[q][:], scalar1=icnt_q[q])

                # meanT via PE transpose, emitted right after mean so the PE
                # wait (coalesced to "all prior DVE ops done") fires earliest
                tp = ps.tile([8, P], F32, tag="psA", bufs=2, space="PSUM", name=f"tp{q}")
                nc.tensor.matmul(out=tp[:], lhsT=mn[q][:], rhs=ident,
                                 is_transpose=True, start=True, stop=True)

                # rneg = -||mean||^2 ; er = exp(rneg)
                msq = sb.tile([P, 8], F32, tag=f"msq{q}", name=f"msq{q}")
                nc.vector.scalar_tensor_tensor(
                    out=msq[:], in0=mn[q][:], scalar=-1.0, in1=mn[q][:],
                    op0=ALU.mult, op1=ALU.mult,
                )
                rneg[q] = sb.tile([P, 1], F32, tag=f"rneg{q}", name=f"rneg{q}")
                nc.vector.reduce_sum(out=rneg[q][:], in_=msq[:], axis=AX.X)
                meant = sb.tile([8, P], F32, tag=f"meant{q}", name=f"meant{q}")
                nc.vector.tensor_copy(out=meant[:], in_=tp[:])
                er[q] = sb.tile([P, 1], F32, tag=f"er{q}", name=f"er{q}")
                nc.scalar.activation(out=er[q][:], in_=rneg[q][:], func=ACT.Exp)

                d2p = ps.tile([P, P], F32, tag="psB", bufs=2, space="PSUM", name=f"d2p{q}")
                nc.tensor.matmul(out=d2p[:], lhsT=meant[:], rhs=meant[:],
                                 start=True, stop=True)

                # ee[i,j] = exp(2G[i,j] - r_i); exp(-r_j) folded into wt below
                ee[q] = sb.tile([P, P], F32, tag=f"ee{q}", name=f"ee{q}")
                nc.scalar.activation(out=ee[q][:], in_=d2p[:], func=ACT.Exp, scale=2.0, bias=rneg[q][:])

            def finish_stats(q):
                nc.vector.reduce_sum(
                    out=s2[:, q:q + 1], in_=s2p[:, 3 * q:3 * q + 3], axis=AX.X,
                )
                # pull: pp = (S2 + cnt*rneg) * icnt / T
                crn = sb.tile([P, 1], F32, tag=f"crn{q}", name=f"crn{q}")
                nc.vector.tensor_tensor(out=crn[:], in0=cnt_q[q], in1=rneg[q][:], op=ALU.mult)
                nc.vector.tensor_tensor(out=crn[:], in0=s2[:, q:q + 1], in1=crn[:], op=ALU.add)
                pp = sb.tile([P, 1], F32, tag=f"pp{q}", name=f"pp{q}")
                nc.vector.tensor_scalar(
                    out=pp[:], in0=crn[:], scalar1=icnt_q[q], scalar2=1.0 / T,
                    op0=ALU.mult, op1=ALU.mult,
                )

                wtq = wt_q[q]
                srhs = sb.tile([P, 12], F32, tag=f"srhs{q}", name=f"srhs{q}")
                nc.vector.tensor_scalar_mul(out=srhs[:, 8:12], in0=wtq, scalar1=er[q][:])
                up = ps.tile([P, TB], F32, tag="psC", bufs=2, space="PSUM", name=f"up{q}")
                nc.tensor.matmul(out=up[:], lhsT=ee[q][:], rhs=wtq,
                                 start=True, stop=True)
                nc.vector.tensor_tensor(out=srhs[:, 0:4], in0=srhs[:, 8:12], in1=up[:], op=ALU.mult)
                nc.vector.tensor_scalar_mul(out=srhs[:, 4:8], in0=bmv, scalar1=pp[:])

                # per-image sums; the -n correction accumulates into the s cols
                nc.tensor.matmul(
                    out=stats_ps[0:1, 8 * q + 4:8 * (q + 1)], lhsT=ones_c, rhs=srhs[:, 4:8],
                    start=True, stop=True,
                )
                nc.tensor.matmul(
                    out=stats_ps[0:1, 8 * q:8 * q + 4], lhsT=ones_c, rhs=wtn_q[q],
                    start=True, stop=False,
                )
                nc.tensor.matmul(
                    out=stats_ps[0:1, 8 * q:8 * q + 4], lhsT=ones_c, rhs=srhs[:, 0:4],
                    start=False, stop=True,
                )

            rneg = [None, None]
            er = [None, None]
            ee = [None, None]
            jn = {0: 0, 1: 0}
            for (q, k0, k1) in CHUNKS:
                chunk_ops(q, k0, k1, jn[q])
                jn[q] += 1
                if jn[q] == 3:
                    finish_core(q)
            finish_stats(0)
            finish_stats(1)

            # final: one op applies both factors, one DMA stores [1, 16]
            sv = stats_ps[:].rearrange("p (q c b) -> p c q b", q=NQ, c=2, b=TB)
            ov = out_sb[:].rearrange("p (q b c) -> p c q b", q=NQ, b=TB, c=2)
            fv = fac.rearrange("p (c q b) -> p c q b", c=2, q=NQ, b=TB)
            nc.vector.tensor_tensor(out=ov, in0=sv, in1=fv, op=ALU.mult)
            nc.sync.dma_start(out_ext[:], out_sb[:])

    nc.compile()
    return nc


_NC_CACHE = {}


def _get_nc():
    if "nc" not in _NC_CACHE:
        _NC_CACHE["nc"] = build_nc()
    return _NC_CACHE["nc"]


def _pack_consts(idx16, msks, vfq):
    """idx16: [NQ,128,136] i16 wrapped; msks: [NQ,128,K,8] f32; vfq: [NQ,TB,M,K]."""
    pk1 = np.zeros((P, PK1_B), dtype=np.uint8)
    pk1[:, 0:272] = idx16[0].view(np.uint8).reshape(P, 272)
    pk1[:, 272:544] = idx16[1].view(np.uint8).reshape(P, 272)
    pk2 = np.zeros((P, PK2_B), dtype=np.uint8)
    pk2[:, 0:544] = np.ascontiguousarray(msks[0]).view(np.uint8).reshape(P, 544)
    pk2[:, 544:1088] = np.ascontiguousarray(msks[1]).view(np.uint8).reshape(P, 544)
    ident = np.eye(P, dtype=np.float32)
    pk2[:, 1088:1600] = ident.view(np.uint8).reshape(P, 512)
    bmv = np.zeros((P, TB), dtype=np.float32)
    for b in range(TB):
        bmv[b * M:(b + 1) * M, b] = 1.0
    pk2[:, 1600:1616] = bmv.view(np.uint8).reshape(P, 16)
    ones = np.ones((P, 1), dtype=np.float32)
    pk2[:, 1616:1620] = ones.view(np.uint8).reshape(P, 4)
    # per-person / per-image normalizers (keypoint metadata only)
    cnt = np.zeros((P, NQ), dtype=np.float32)
    fac = np.zeros(2 * BL, dtype=np.float32)
    wt = np.zeros((P, NQ * TB), dtype=np.float32)
    for q in range(NQ):
        cq = vfq[q].sum(axis=2).reshape(PU)          # [120]
        cnt[:PU, q] = cq
        h = np.minimum(cq, 1.0)
        wt[:PU, TB * q:TB * (q + 1)] = bmv[:PU] * h[:, None]
        n = h.reshape(TB, M).sum(axis=1)             # [4]
        iq = 0.5 * np.clip(n - 1.0, 0.0, 1.0) / np.maximum(n * (n - 1.0), 1.0)
        ipn = 1.0 / np.maximum(n, 1.0)
        fac[0 * BL + TB * q:0 * BL + TB * (q + 1)] = iq
        fac[1 * BL + TB * q:1 * BL + TB * (q + 1)] = ipn
    icnt = 1.0 / np.maximum(cnt, 1.0)
    pk2[:, 1620:1628] = cnt.view(np.uint8).reshape(P, 8)
    pk2[:, 1628:1636] = icnt.view(np.uint8).reshape(P, 8)
    pk2[:, 1636:1668] = wt.view(np.uint8).reshape(P, 32)
    pk2[:, 1668:1700] = (-wt).view(np.uint8).reshape(P, 32)
    pk2[0, 1700:1764] = fac.view(np.uint8)
    return pk1, pk2


def make_in_maps(tags, keypoints):
    tags = np.asarray(tags, dtype=np.float32)
    kp = np.asarray(keypoints)
    idx = np.clip(kp[..., 0].astype(np.int64), 0, N - 1)   # [B, M, K]
    vf = (kp[..., 1] > 0).astype(np.float32)               # [B, M, K]

    in_maps = []
    for c in range(NCORES):
        halves = []
        idx16 = np.zeros((NQ, P, 136), dtype=np.int16)
        msks = np.zeros((NQ, P, K, 8), dtype=np.float32)
        vfq = np.zeros((NQ, TB, M, K), dtype=np.float32)
        for q in range(NQ):
            sl = slice(BL * c + TB * q, BL * c + TB * (q + 1))
            halves.append(np.ascontiguousarray(tags[sl].reshape(NBLK, 64)))
            iq_ = idx[sl]   # [TB, M, K]
            vq = vf[sl]
            vfq[q] = vq
            # flat row within half -> block and sub-row
            rows = (np.arange(TB, dtype=np.int64)[:, None, None] * N + iq_)  # [TB, M, K]
            blk_q = (rows >> 3).astype(np.int16)      # [TB, M, K] in [0, 32768)
            sub_q = (rows & 7).astype(np.int64)
            # item (slot k, partition p): p = img*M + person; wrapped idx
            # layout per gather chunk (slots 0-7 / 8-15 / 16)
            pidx = np.arange(PU)
            img, per = pidx // M, pidx % M
            col = 0
            for (k0, k1) in ((0, 8), (8, 16), (16, 17)):
                ni = (k1 - k0) * P
                vals = np.zeros(ni, dtype=np.int16)
                for k in range(k0, k1):
                    vals[(k - k0) * P + pidx] = blk_q[img, per, k]
                wrapped = vals.reshape(ni // 16, 16).T   # [16, ni/16]
                idx16[q][:, col:col + ni // 16] = np.tile(wrapped, (8, 1))
                col += ni // 16
            # masks
            mq = np.zeros((P, K, 8), dtype=np.float32)
            mq[pidx[:, None], np.arange(K)[None, :], sub_q[img, per, :]] = vq[img, per, :]
            msks[q] = mq
        pk1, pk2 = _pack_consts(idx16, msks, vfq)
        in_maps.append({
            "tags_a": halves[0],
            "tags_b": halves[1],
            "pk1": pk1,
            "pk2": pk2,
        })
    return in_maps


def kernel(tags, keypoints):
    nc = _get_nc()
    in_maps = make_in_maps(tags, keypoints)
    last_err = None
    for _attempt in range(3):
        try:
            res = run_bass_kernel_spmd(nc, in_maps, core_ids=list(range(NCORES))).results
            break
        except Exception as e:  # a crashed predecessor can leave the NC wedged;
            last_err = e        # the failed attempt clears it, so retry
            import time
            time.sleep(1.0)
    else:
        raise last_err
    out = np.concatenate([res[c]["out"].reshape(BL, 2) for c in range(NCORES)], axis=0)
    return out.astype(np.float32)


# revision 17
# speedup vs baseline: 1.0133x; 1.0133x over previous
"""Associative-embedding (AE) loss on 8 TRN2 NeuronCores, data-parallel over batch.

Reference computation (per batch image b):
  g[m,k,:]   = tags[b, idx[b,m,k], :]                       (gather, T=8)
  mean[m,:]  = sum_k vf*g / max(cnt,1)                      (cnt = sum_k vf)
  pull       = (1/max(n,1)) * sum_m (1/max(cnt,1)) * sum_k vf * mean_t (g-mean)^2
  push       = 0.5/max(n(n-1),1) * sum_{i!=j valid} exp(-||mean_i-mean_j||^2)  (if n>1)
  out[b]     = [push, pull]

Sharding: batch dim B=64 split across 8 cores (8 images each); all reductions
are batch-local, no collectives; host concatenates per-core outputs.

Gather strategy: instead of 34 per-joint indirect DMAs (~1us of Pool SWDGE
descriptor-generation each), use TWO InstDMAGatherAnt instructions, one per
4-image quad. Each consumes int16 *block* indices and fetches the 256-byte
block (8 tag rows) containing each joint's row:
  - per-core tags are viewed as two [32768, 64] f32 halves (4 images each) so
    block indices fit int16's positive range,
  - item (slot k, partition p) of a gather lands at out[p, k, 0:64]; we place
    persons on partitions (120 of 128 used) and joints on slots,
  - the 8->1 sub-row selection is done on-chip with host-built masks
    M[p,k,s] = vf * (row & 7 == s), broadcast over the tag dim via a
    stride-0 AP, fused into one tensor_tensor_reduce per quad (gm = blk * M),
  - per-person sums then never need per-joint tensors: S1 = sum_{k,s} gm
    (DVE reduce keeping t), S2 = sum gm*blk = sum vf*g^2 (second ttr).
The remaining tail matches the old kernel: mean -> PE transpose -> Gram ->
exp(2G - r_i) with row-norm bias -> masked matmuls for push, and
pull = (S2 + cnt*rneg)/(T*cnt); n-derived factors come from the masks alone
and complete during the gather window.
"""

import numpy as np

import concourse.bass as bass
import concourse.tile as tile
from concourse import bacc, mybir
from concourse.bass_utils import run_bass_kernel_spmd
from concourse.tile_rust import add_dep_helper

B, N, T = 64, 65536, 8
M, K = 30, 17
NCORES = 8
BL = B // NCORES   # images per core
TB = 4             # images per quad
NQ = BL // TB      # quads per core (2)
P = 128            # partitions (TB*M = 120 used)
PU = TB * M        # used partitions
NI = K * P         # num_idxs per quad gather (2176)
NBLK = TB * N // 8  # 32768 blocks per tags half
F32 = mybir.dt.float32
F32R = mybir.dt.float32r
I16 = mybir.dt.int16
U8 = mybir.dt.uint8

ALU = mybir.AluOpType
AX = mybir.AxisListType
ACT = mybir.ActivationFunctionType

# packed-constant byte layout (per partition) for the two input DMAs
#   pk1: [idxA (272B) | idxB (272B)]                      -> needed first
#   pk2: [M_A (544B) | M_B (544B) | ident (512B) | bmv(16B) | ones(4B) |
#         cnt(8B) | icnt(8B) | wt(32B) | wtn(32B) | fac(64B, partition 0)]
PK1_B = 544
PK2_B = 544 + 544 + 512 + 16 + 4 + 8 + 8 + 32 + 32 + 64


def build_nc():
    nc = bacc.Bacc("TRN2", target_bir_lowering=False, debug=False, num_devices=NCORES)
    tags_a = nc.declare_dram_parameter("tags_a", [NBLK, 64], F32, isOutput=False)
    tags_b = nc.declare_dram_parameter("tags_b", [NBLK, 64], F32, isOutput=False)
    pk1_ext = nc.declare_dram_parameter("pk1", [P, PK1_B], U8, isOutput=False)
    pk2_ext = nc.declare_dram_parameter("pk2", [P, PK2_B], U8, isOutput=False)
    out_ext = nc.declare_dram_parameter("out", [1, BL * 2], F32, isOutput=True)

    with tile.TileContext(nc) as tc:
        with (
            tc.tile_pool(name="sb", bufs=1) as sb,
            tc.tile_pool(name="ps", bufs=1, space="PSUM") as ps,
        ):
            pk1 = sb.tile([P, PK1_B], U8, tag="pk1")
            nc.sync.dma_start(pk1[:], pk1_ext[:])
            pk2 = sb.tile([P, PK2_B], U8, tag="pk2")
            nc.scalar.dma_start(pk2[:], pk2_ext[:])

            idx = [pk1[:, 0:272].bitcast(I16), pk1[:, 272:544].bitcast(I16)]
            msk = [
                pk2[:, 0:544].bitcast(F32).rearrange("p (k s) -> p k s", s=8),
                pk2[:, 544:1088].bitcast(F32).rearrange("p (k s) -> p k s", s=8),
            ]
            ident = pk2[:, 1088:1600].bitcast(F32)
            bmv = pk2[:, 1600:1616].bitcast(F32)
            ones_c = pk2[:, 1616:1620].bitcast(F32)
            cnt_q = [pk2[:, 1620 + 4 * q:1624 + 4 * q].bitcast(F32) for q in range(NQ)]
            icnt_q = [pk2[:, 1628 + 4 * q:1632 + 4 * q].bitcast(F32) for q in range(NQ)]
            wt_q = [pk2[:, 1636 + 16 * q:1652 + 16 * q].bitcast(F32) for q in range(NQ)]
            wtn_q = [pk2[:, 1668 + 16 * q:1684 + 16 * q].bitcast(F32) for q in range(NQ)]
            fac = pk2[0:1, 1700:1764].bitcast(F32)

            srcs = [tags_a, tags_b]
            blk = [sb.tile([P, K, 64], F32, tag=f"blk{q}", name=f"blk{q}") for q in range(NQ)]
            gm = [sb.tile([P, K, 8, 8], F32, tag=f"gm{q}", name=f"gm{q}") for q in range(NQ)]
            sq = sb.tile([P, K * 64], F32, tag="sq")
            stats_ps = ps.tile([1, NQ * 8], F32, tag="stats", bufs=1, space="PSUM")
            out_sb = sb.tile([1, BL * 2], F32, tag="out_sb")

            # quad gathers, chunked: HW caps one dma_gather at 1024 indices,
            # so each quad is 3 instructions (slots 0-7 / 8-15 / 16).
            # Order A1 A2 B1 A3 B2 B3: quad A completes early (its whole tail
            # hides under quad B's transfers) and the bus stays near-saturated.
            CHUNKS = [(0, 0, 8), (0, 8, 16), (1, 0, 8), (0, 16, 17), (1, 8, 16), (1, 16, 17)]
            CHUNK_OFF = {(0, 8): 0, (8, 16): 128, (16, 17): 256}
            for (q, k0, k1) in CHUNKS:
                ob = 272 * q + CHUNK_OFF[(k0, k1)]
                ni = (k1 - k0) * P
                nc.gpsimd.dma_gather(
                    out_ap=blk[q][:, k0:k1, :],
                    in_ap=srcs[q][:],
                    idxs_ap=pk1[:, ob:ob + ni // 8].bitcast(I16),
                    num_idxs=ni,
                    num_idxs_reg=ni,
                    elem_size=64,
                )

            # per-quad pipeline, processed per gather chunk in arrival order so
            # the DVE/ACT streams never head-of-line block on late data
            s2p = sb.tile([P, NQ * 3], F32, tag="s2p")
            s2 = sb.tile([P, NQ], F32, tag="s2")
            s1p = [sb.tile([P, 24], F32, tag=f"s1p{q}", name=f"s1p{q}") for q in range(NQ)]
            s1 = [sb.tile([P, 8], F32, tag=f"s1{q}", name=f"s1{q}") for q in range(NQ)]
            mn = [sb.tile([P, 8], F32, tag=f"mn{q}", name=f"mn{q}") for q in range(NQ)]

            def chunk_ops(q, k0, k1, j):
                nk = k1 - k0
                gm_s = gm[q][:, k0:k1]
                nc.vector.tensor_tensor(
                    out=gm_s,
                    in0=blk[q][:, k0:k1, :].rearrange("p k (s t) -> p k s t", t=8),
                    in1=msk[q][:, k0:k1].broadcast_to((P, nk, 8, 8)),
                    op=ALU.mult,
                )
                nc.vector.reduce_sum(
                    out=s1p[q][:, 8 * j:8 * (j + 1)],
                    in_=gm_s.rearrange("p k s t -> p t (k s)"),
                    axis=AX.X,
                )
                nc.scalar.activation(
                    out=sq[:, k0 * 64:k1 * 64],
                    in_=gm_s.rearrange("p k s t -> p (k s t)"),
                    func=ACT.Square, accum_out=s2p[:, 3 * q + j:3 * q + j + 1],
                )

            def finish_core(q):
                sp = s1p[q]
                nc.vector.tensor_tensor(out=s1[q][:], in0=sp[:, 0:8], in1=sp[:, 8:16], op=ALU.add)
                nc.vector.tensor_tensor(out=s1[q][:], in0=s1[q][:], in1=sp[:, 16:24], op=ALU.add)
                nc.vector.tensor_scalar_mul(out=mn[q][:], in0=s1[q][:], scalar1=icnt_q[q])

                # meanT via PE transpose, emitted right after mean so the PE
                # wait (coalesced to "all prior DVE ops done") fires earliest
                tp = ps.tile([8, P], F32, tag="psA", bufs=2, space="PSUM", name=f"tp{q}")
                nc.tensor.matmul(out=tp[:], lhsT=mn[q][:], rhs=ident,
                                 is_transpose=True, start=True, stop=True)

                # rneg = -||mean||^2 ; er = exp(rneg)
                msq = sb.tile([P, 8], F32, tag=f"msq{q}", name=f"msq{q}")
                nc.vector.scalar_tensor_tensor(
                    out=msq[:], in0=mn[q][:], scalar=-1.0, in1=mn[q][:],
                    op0=ALU.mult, op1=ALU.mult,
                )
                rneg[q] = sb.tile([P, 1], F32, tag=f"rneg{q}", name=f"rneg{q}")
                nc.vector.reduce_sum(out=rneg[q][:], in_=msq[:], axis=AX.X)
                meant = sb.tile([8, P], F32, tag=f"meant{q}", name=f"meant{q}")
                nc.vector.tensor_copy(out=meant[:], in_=tp[:])
                er[q] = sb.tile([P, 1], F32, tag=f"er{q}", name=f"er{q}")
                nc.scalar.activation(out=er[q][:], in_=rneg[q][:], func=ACT.Exp)

                d2p = ps.tile([P, P], F32, tag="psB", bufs=2, space="PSUM", name=f"d2p{q}")
                nc.tensor.matmul(out=d2p[:], lhsT=meant[:], rhs=meant[:],
                                 start=True, stop=True)

                # ee[i,j] = exp(2G[i,j] - r_i); exp(-r_j) folded into wt below
                ee[q] = sb.tile([P, P], F32, tag=f"ee{q}", name=f"ee{q}")
                nc.scalar.activation(out=ee[q][:], in_=d2p[:], func=ACT.Exp, scale=2.0, bias=rneg[q][:])

            def finish_stats(q):
                nc.vector.reduce_sum(
                    out=s2[:, q:q + 1], in_=s2p[:, 3 * q:3 * q + 3], axis=AX.X,
                )
                # pull: pp = (S2 + cnt*rneg) * icnt / T
                crn = sb.tile([P, 1], F32, tag=f"crn{q}", name=f"crn{q}")
                nc.vector.tensor_tensor(out=crn[:], in0=cnt_q[q], in1=rneg[q][:], op=ALU.mult)
                nc.vector.tensor_tensor(out=crn[:], in0=s2[:, q:q + 1], in1=crn[:], op=ALU.add)
                pp = sb.tile([P, 1], F32, tag=f"pp{q}", name=f"pp{q}")
                nc.vector.tensor_scalar(
                    out=pp[:], in0=crn[:], scalar1=icnt_q[q], scalar2=1.0 / T,
                    op0=ALU.mult, op1=ALU.mult,
                )

                wtq = wt_q[q]
                srhs = sb.tile([P, 12], F32, tag=f"srhs{q}", name=f"srhs{q}")
                nc.vector.tensor_scalar_mul(out=srhs[:, 8:12], in0=wtq, scalar1=er[q][:])
                up = ps.tile([P, TB], F32, tag="psC", bufs=2, space="PSUM", name=f"up{q}")
                nc.tensor.matmul(out=up[:], lhsT=ee[q][:], rhs=wtq,
                                 start=True, stop=True)
                nc.vector.tensor_tensor(out=srhs[:, 0:4], in0=srhs[:, 8:12], in1=up[:], op=ALU.mult)
                nc.vector.tensor_scalar_mul(out=srhs[:, 4:8], in0=bmv, scalar1=pp[:])

                # per-image sums; the -n correction accumulates into the s cols
                nc.tensor.matmul(
                    out=stats_ps[0:1, 8 * q + 4:8 * (q + 1)], lhsT=ones_c, rhs=srhs[:, 4:8],
                    start=True, stop=True,
                )
                nc.tensor.matmul(
                    out=stats_ps[0:1, 8 * q:8 * q + 4], lhsT=ones_c, rhs=wtn_q[q],
                    start=True, stop=False,
                )
                nc.tensor.matmul(
                    out=stats_ps[0:1, 8 * q:8 * q + 4], lhsT=ones_c, rhs=srhs[:, 0:4],
                    start=False, stop=True,
                )

            rneg = [None, None]
            er = [None, None]
            ee = [None, None]
            jn = {0: 0, 1: 0}
            for (q, k0, k1) in CHUNKS:
                chunk_ops(q, k0, k1, jn[q])
                jn[q] += 1
                if jn[q] == 3:
                    finish_core(q)
            finish_stats(0)
            finish_stats(1)

            # final: one op applies both factors, one DMA stores [1, 16]
            sv = stats_ps[:].rearrange("p (q c b) -> p c q b", q=NQ, c=2, b=TB)
            ov = out_sb[:].rearrange("p (q b c) -> p c q b", q=NQ, b=TB, c=2)
            fv = fac.rearrange("p (c q b) -> p c q b", c=2, q=NQ, b=TB)
            nc.vector.tensor_tensor(out=ov, in0=sv, in1=fv, op=ALU.mult)
            nc.sync.dma_start(out_ext[:], out_sb[:])

    nc.compile()
    return nc


_NC_CACHE = {}


def _get_nc():
    if "nc" not in _NC_CACHE:
        _NC_CACHE["nc"] = build_nc()
    return _NC_CACHE["nc"]


def _pack_consts(idx16, msks, vfq):
    """idx16: [NQ,128,136] i16 wrapped; msks: [NQ,128,K,8] f32; vfq: [NQ,TB,M,K]."""
    pk1 = np.zeros((P, PK1_B), dtype=np.uint8)
    pk1[:, 0:272] = idx16[0].view(np.uint8).reshape(P, 272)
    pk1[:, 272:544] = idx16[1].view(np.uint8).reshape(P, 272)
    pk2 = np.zeros((P, PK2_B), dtype=np.uint8)
    pk2[:, 0:544] = np.ascontiguousarray(msks[0]).view(np.uint8).reshape(P, 544)
    pk2[:, 544:1088] = np.ascontiguousarray(msks[1]).view(np.uint8).reshape(P, 544)
    ident = np.eye(P, dtype=np.float32)
    pk2[:, 1088:1600] = ident.view(np.uint8).reshape(P, 512)
    bmv = np.zeros((P, TB), dtype=np.float32)
    for b in range(TB):
        bmv[b * M:(b + 1) * M, b] = 1.0
    pk2[:, 1600:1616] = bmv.view(np.uint8).reshape(P, 16)
    ones = np.ones((P, 1), dtype=np.float32)
    pk2[:, 1616:1620] = ones.view(np.uint8).reshape(P, 4)
    # per-person / per-image normalizers (keypoint metadata only)
    cnt = np.zeros((P, NQ), dtype=np.float32)
    fac = np.zeros(2 * BL, dtype=np.float32)
    wt = np.zeros((P, NQ * TB), dtype=np.float32)
    for q in range(NQ):
        cq = vfq[q].sum(axis=2).reshape(PU)          # [120]
        cnt[:PU, q] = cq
        h = np.minimum(cq, 1.0)
        wt[:PU, TB * q:TB * (q + 1)] = bmv[:PU] * h[:, None]
        n = h.reshape(TB, M).sum(axis=1)             # [4]
        iq = 0.5 * np.clip(n - 1.0, 0.0, 1.0) / np.maximum(n * (n - 1.0), 1.0)
        ipn = 1.0 / np.maximum(n, 1.0)
        fac[0 * BL + TB * q:0 * BL + TB * (q + 1)] = iq
        fac[1 * BL + TB * q:1 * BL + TB * (q + 1)] = ipn
    icnt = 1.0 / np.maximum(cnt, 1.0)
    pk2[:, 1620:1628] = cnt.view(np.uint8).reshape(P, 8)
    pk2[:, 1628:1636] = icnt.view(np.uint8).reshape(P, 8)
    pk2[:, 1636:1668] = wt.view(np.uint8).reshape(P, 32)
    pk2[:, 1668:1700] = (-wt).view(np.uint8).reshape(P, 32)
    pk2[0, 1700:1764] = fac.view(np.uint8)
    return pk1, pk2


def make_in_maps(tags, keypoints):
    tags = np.asarray(tags, dtype=np.float32)
    kp = np.asarray(keypoints)
    idx = np.clip(kp[..., 0].astype(np.int64), 0, N - 1)   # [B, M, K]
    vf = (kp[..., 1] > 0).astype(np.float32)               # [B, M, K]

    in_maps = []
    for c in range(NCORES):
        halves = []
        idx16 = np.zeros((NQ, P, 136), dtype=np.int16)
        msks = np.zeros((NQ, P, K, 8), dtype=np.float32)
        vfq = np.zeros((NQ, TB, M, K), dtype=np.float32)
        for q in range(NQ):
            sl = slice(BL * c + TB * q, BL * c + TB * (q + 1))
            halves.append(np.ascontiguousarray(tags[sl].reshape(NBLK, 64)))
            iq_ = idx[sl]   # [TB, M, K]
            vq = vf[sl]
            vfq[q] = vq
            # flat row within half -> block and sub-row
            rows = (np.arange(TB, dtype=np.int64)[:, None, None] * N + iq_)  # [TB, M, K]
            blk_q = (rows >> 3).astype(np.int16)      # [TB, M, K] in [0, 32768)
            sub_q = (rows & 7).astype(np.int64)
            # item (slot k, partition p): p = img*M + person; wrapped idx
            # layout per gather chunk (slots 0-7 / 8-15 / 16)
            pidx = np.arange(PU)
            img, per = pidx // M, pidx % M
            col = 0
            for (k0, k1) in ((0, 8), (8, 16), (16, 17)):
                ni = (k1 - k0) * P
                vals = np.zeros(ni, dtype=np.int16)
                for k in range(k0, k1):
                    vals[(k - k0) * P + pidx] = blk_q[img, per, k]
                wrapped = vals.reshape(ni // 16, 16).T   # [16, ni/16]
                idx16[q][:, col:col + ni // 16] = np.tile(wrapped, (8, 1))
                col += ni // 16
            # masks
            mq = np.zeros((P, K, 8), dtype=np.float32)
            mq[pidx[:, None], np.arange(K)[None, :], sub_q[img, per, :]] = vq[img, per, :]
            msks[q] = mq
        pk1, pk2 = _pack_consts(idx16, msks, vfq)
        in_maps.append({
            "tags_a": halves[0],
            "tags_b": halves[1],
            "pk1": pk1,
            "pk2": pk2,
        })
    return in_maps


def kernel(tags, keypoints):
    nc = _get_nc()
    in_maps = make_in_maps(tags, keypoints)
    last_err = None
    for _attempt in range(3):
        try:
            res = run_bass_kernel_spmd(nc, in_maps, core_ids=list(range(NCORES))).results
            break
        except Exception as e:  # a crashed predecessor can leave the NC wedged;
            last_err = e        # the failed attempt clears it, so retry
            import time
            time.sleep(1.0)
    else:
        raise last_err
    out = np.concatenate([res[c]["out"].reshape(BL, 2) for c in range(NCORES)], axis=0)
    return out.astype(np.float32)
